# revision 67
# baseline (speedup 1.0000x reference)
"""Trainium2 Bass kernel for nn_ExplainerCompatibleGinGru.

Math: the reference pads the batch with 31 zero graphs, splits the node dim
into two 36-node graphs (ad = rows 0:36, dis = rows 36:72), runs 3 GIN layers
with sum-pooling, packs [ad x (L-1), dis] as a GRU sequence per batch
element, and returns out[0] -- which depends ONLY on graph 0 (ad), graph 32
(dis) and L = LOS_batch[0].  So the kernel computes: GIN on the stacked
72-node 2-graph block, an L-step GRU on one sequence, and a tiny classifier.

Runs replicated on all 8 cores.  PE matmuls are fp16 with fp32 PSUM
accumulation; LN/gates run in fp32.

Key structure vs the naive version:
- DMA is issued in priority order across all three DMA-capable queues
  (sync HWDGE, act HWDGE, gpsimd SWDGE) so compute is never weight-starved:
  layer-1 weights first, then layer-2/3, whht (GRU), wiht (gi), wc1 last.
- aggregation is a dense matmul with the (A+I)^T block-diagonal operator;
  the GIN-MLP bias rides along as a 73rd "homogeneous" row of z (ones
  column appended to the aggregation operator) -- no bias matmuls.
- LN apply+transpose fuse: us = (u-mean)*rstd, PE transpose, then
  relu(g*x+be) per chunk split across ACT/DVE/GPSIMD.
- GRU per-step: psum tiles are pre-initialized with the gate biases by
  gpsimd (no closer matmuls); the z-gate weights and biases are negated
  host-side so (1-z) comes straight out of one sigmoid:
    h' = w*(n - h) + h   with w = sigmoid(-(Whh_z h + b_z + gi_z)).
- ACT tables (Rsqrt/Sigmoid/Tanh) are prefetched at t=0.
"""

import os
import numpy as np
import ml_dtypes  # noqa: F401

F16 = np.float16

H = 512
LN_EPS = 1e-5

_prog_cache = {}
last_run_info = {}


def _pack_kchunks(w, ncols):
    """[K, N] weight -> [128, (K//128)*N], chunk kc at cols [N*kc, N*(kc+1))."""
    k, n = w.shape
    assert k % 128 == 0 and n == ncols
    nk = k // 128
    return np.ascontiguousarray(
        w.reshape(nk, 128, n).transpose(1, 0, 2).reshape(128, nk * n))


def _prep_inputs(inputs):
    f32 = np.float32

    def bf(x):
        return np.asarray(x, f32).astype(F16)

    x = np.asarray(inputs['x_embedded'], f32)
    tei = np.asarray(inputs['template_edge_index']).astype(np.int64)
    L = int(np.asarray(inputs['LOS_batch']).reshape(-1)[0])

    A = np.zeros((36, 36), f32)
    np.add.at(A, (tei[1], tei[0]), 1.0)
    Mp = A + np.eye(36, dtype=f32)
    m72x = np.zeros((72, 72), f32)
    m72x[:36, :36] = Mp.T
    m72x[36:, 36:] = Mp.T

    W = {k: np.asarray(v, f32) for k, v in inputs.items()
         if k not in ('x_embedded', 'template_edge_index', 'LOS_batch')}

    # xe0a: x0 + eye72  [72, 104]
    xe0a = np.zeros((72, 104), F16)
    xe0a[:, 0:32] = bf(x)
    xe0a[:, 32:104] = np.eye(72, dtype=F16)

    # xe0b: m72 | w1a (rows 0:32) | ba0 | ba1 (row 0)  [72, 1608]
    xe0b = np.zeros((72, 1608), F16)
    xe0b[:, 0:72] = bf(m72x)
    xe0b[0:32, 72:584] = bf(W['W1a'])
    xe0b[0, 584:1096] = bf(W['b1a'])
    xe0b[0, 1096:1608] = bf(W['bha'])

    # fv layout (f32):
    #  0:24  combo24: [p, 2j+g] = bih[p+128j] (+ bhh[p+128j] for j<8);
    #        z-gate cols (j 4..7) NEGATED
    # 24:28  bhh_n tile; 28 bc2; 29:33 g1T; 33:37 be1T; 37:41 ghT; 41:45 behT
    # 45:49  b1bT; 49:53 bhbT; 53:61 bc1T
    fv = np.zeros((128, 61), f32)
    bih_t = W['bih'].reshape(12, 128).T
    bhh_t = W['bhh'].reshape(12, 128).T
    combo = bih_t.copy()
    combo[:, 0:8] += bhh_t[:, 0:8]
    combo[:, 4:8] *= -1.0  # z gates negated
    fv[:, 0:24:2] = combo
    fv[:, 1:24:2] = combo
    fv[:, 24:28] = bhh_t[:, 8:12]
    fv[:, 28] = W['bc2'][0]
    fv[:, 29:33] = W['g1'].reshape(4, 128).T
    fv[:, 33:37] = W['be1'].reshape(4, 128).T
    fv[:, 37:41] = W['gh'].reshape(4, 128).T
    fv[:, 41:45] = W['beh'].reshape(4, 128).T
    fv[:, 45:49] = W['b1b'].reshape(4, 128).T
    fv[:, 49:53] = W['bhb'].reshape(4, 128).T
    fv[:, 53:61] = W['bc1'].reshape(8, 128).T

    # whht: [128, 4*1536], chunk c at cols [1536c, 1536c+1536), j slice
    # 128j within; z-gate columns (j 4..7) negated for the (1-z) trick
    whht = _pack_kchunks(np.ascontiguousarray(W['Whh'].T), 1536)
    for c in range(4):
        whht[:, 1536 * c + 512:1536 * c + 1024] *= -1.0

    w1b = _pack_kchunks(W['W1b'], H)     # [128, 2048]
    wha = _pack_kchunks(W['Wha'], H)
    whb = _pack_kchunks(W['Whb'], H)
    wiht = _pack_kchunks(np.ascontiguousarray(W['Wih'].T), 1536)  # [128,18432]
    for kc in range(12):  # negate z-gate output cols (j 4..7), as in whht
        wiht[:, 1536 * kc + 512:1536 * kc + 1024] *= -1.0
    wc1 = _pack_kchunks(W['Wc1'], 1024)  # [128, 4096]

    # misc16: wc2 | eye128 | bhn16 | bc116  [128, 148]
    misc16 = np.zeros((128, 148), F16)
    misc16[:, 0:8] = bf(np.ascontiguousarray(W['Wc2'].reshape(8, 128).T))
    misc16[:, 8:136] = np.eye(128, dtype=F16)
    misc16[:, 136:140] = bf(bhh_t[:, 8:12])
    misc16[:, 140:148] = bf(W['bc1'].reshape(8, 128).T)

    blobs = {
        'xe0a': xe0a,
        'xe0b': xe0b,
        'fv': fv,
        'misc16': misc16,
        'w1b16': bf(w1b),
        'wha16': bf(wha),
        'whb16': bf(whb),
        'whht0': bf(whht[:, 0:1536]).copy(),
        'whht1': bf(whht[:, 1536:3072]).copy(),
        'whht2': bf(whht[:, 3072:4608]).copy(),
        'whht3': bf(whht[:, 4608:6144]).copy(),
        'wc1a': bf(wc1[:, 0:2048]).copy(),
        'wc1b': bf(wc1[:, 2048:4096]).copy(),
    }
    for i in range(6):
        blobs[f'wiht{i}'] = bf(wiht[:, 3072 * i:3072 * (i + 1)]).copy()
    return blobs, L


def _emit(ctx, tc, d, out_dram, L):
    import concourse.mybir as mybir
    nc = tc.nc
    f32 = mybir.dt.float32
    f16 = mybir.dt.float16
    AF = mybir.ActivationFunctionType
    AL = mybir.AluOpType

    wts = ctx.enter_context(tc.tile_pool(name="wts", bufs=1))
    act = ctx.enter_context(tc.tile_pool(name="act", bufs=1))
    tmp = ctx.enter_context(tc.tile_pool(name="tmp", bufs=2))
    from contextlib import ExitStack
    pgi = ctx.enter_context(tc.tile_pool(name="pgi", bufs=1, space="PSUM"))
    gin_ctx = ctx.enter_context(ExitStack())
    pbig = gin_ctx.enter_context(tc.tile_pool(name="pbig", bufs=2,
                                              space="PSUM"))
    psm = gin_ctx.enter_context(tc.tile_pool(name="psm", bufs=2,
                                             space="PSUM"))

    # ---- DMA issue, priority order across the 3 queues ----------------
    def T(pool, name, shape, dtype=f16):
        t = pool.tile(list(shape), dtype, tag=name, name=name)
        return t

    # Ring model: ~2us fixed cost per entry, FIFO per ring, 3 rings
    # (sync HWDGE / act HWDGE / gpsimd SWDGE) running concurrently.
    # act ring gets exactly TWO entries pushed up-front (a deeper list
    # could stall the ACT sequencer on ring-full and block compute).
    xe0a = T(wts, 'xe0a', (72, 104))
    nc.sync.dma_start(xe0a[:, :], d['xe0a'])
    xe0b = T(wts, 'xe0b', (72, 1608))
    nc.sync.dma_start(xe0b[:, :], d['xe0b'])
    fv = T(wts, 'fv', (128, 61), f32)
    nc.sync.dma_start(fv[:, :], d['fv'])
    misc16 = T(wts, 'misc16', (128, 148))
    nc.sync.dma_start(misc16[:, :], d['misc16'])
    m72x = xe0b[:, 0:72]
    w1a16 = xe0b[0:32, 72:584]
    ba_sb = [xe0b[0:1, 584:1096], xe0b[0:1, 1096:1608]]
    wc2 = misc16[:, 0:8]
    eye128 = misc16[:, 8:136]
    bhn16 = misc16[:, 136:140]
    bc116 = misc16[:, 140:148]
    ones72 = act.tile([1, 72], f16, tag='ones72')
    nc.vector.memset(ones72[:, :], 1.0)

    # GIN MLP weights: all on the fast gpsimd ring (3 entries, in the
    # order the layers consume them)
    w1b16 = T(wts, 'w1b16', (128, 2048))
    wha16 = T(wts, 'wha16', (128, 2048))
    whb16 = T(wts, 'whb16', (128, 2048))
    nc.gpsimd.dma_start(w1b16[:, :], d['w1b16'])
    nc.gpsimd.dma_start(wha16[:, :], d['wha16'])
    nc.gpsimd.dma_start(whb16[:, :], d['whb16'])

    # sync ring: two wiht chunks early (gi gates the GRU start), then
    # whht halves; gpsimd carries the rest
    wiht_t = [T(wts, f'wiht{i}', (128, 3072)) for i in range(6)]
    whht_t = [T(wts, f'whht{c}', (128, 1536)) for c in range(4)]
    nc.sync.dma_start(wiht_t[0][:, :], d['wiht0'])
    nc.sync.dma_start(wiht_t[3][:, :], d['wiht3'])
    nc.sync.dma_start(whht_t[0][:, :], d['whht0'])
    nc.sync.dma_start(whht_t[2][:, :], d['whht2'])
    nc.gpsimd.dma_start(wiht_t[1][:, :], d['wiht1'])
    nc.gpsimd.dma_start(wiht_t[4][:, :], d['wiht4'])
    nc.gpsimd.dma_start(whht_t[1][:, :], d['whht1'])
    nc.gpsimd.dma_start(whht_t[3][:, :], d['whht3'])
    nc.scalar.dma_start(wiht_t[2][:, :], d['wiht2'])
    nc.scalar.dma_start(wiht_t[5][:, :], d['wiht5'])

    wc1t = [T(wts, 'wc1a', (128, 2048)), T(wts, 'wc1b', (128, 2048))]
    nc.sync.dma_start(wc1t[0][:, :], d['wc1a'])
    nc.gpsimd.dma_start(wc1t[1][:, :], d['wc1b'])

    def wiht_chunk(kc, j):
        q, r = divmod(kc, 2)
        return wiht_t[q][:, 1536 * r + 128 * j:1536 * r + 128 * j + 128]

    def whht_chunk(c, j):
        return whht_t[c][:, 128 * j:128 * (j + 1)]

    # ---- ACT table prefetches (Rsqrt, Sigmoid, Tanh) ------------------
    sc1 = act.tile([1, 1], f32, tag='sc1')
    nc.vector.memset(sc1[:, :], 1.0)
    eps = act.tile([72, 1], f32, tag='eps')
    nc.vector.memset(eps[:, :], LN_EPS)
    for i, af in enumerate((AF.Identity, AF.Sqrt, AF.Sigmoid, AF.Tanh)):
        scd = act.tile([1, 1], f32, tag=f'scd{i}')
        nc.scalar.activation(scd[:, :], sc1[:, :], af)

    x0s = xe0a[:, 0:32]
    eye72 = xe0a[:, 32:104]

    # per-layer feats tiles: finer deps let layer-l gi matmuls run as
    # soon as layer l's pooling (and the wiht chunk) lands, instead of
    # waiting for all three layers
    featsT_l = [act.tile([128, 8], f16, tag=f'featsT{l}', name=f'featsT{l}')
                for l in range(3)]
    gi_ps = pgi.tile([128, 24], f32, tag='gi')

    # ---- GIN layers (activations live feature-major between layers) ----
    x0T = tmp.tile([32, 72], f16, tag='x0T')
    tp0 = psm.tile([128, 72], f16, tag='psm')
    nc.tensor.transpose(tp0[0:32, :], x0s, eye72)
    nc.vector.tensor_copy(x0T[:, :], tp0[0:32, :])

    gi_backlog = {}
    hT_chunks = [x0T]   # list of [<=128, 72] feature-major chunks
    for l in range(3):
        wa_t = w1a16 if l == 0 else wha16
        wb_t = w1b16 if l == 0 else whb16
        gcol = 29 if l == 0 else 37
        becol = 33 if l == 0 else 41
        bbtcol = 45 if l == 0 else 49
        nk = len(hT_chunks)

        def wa_slice(kc, q):
            # [128, 256] slice for k-chunk kc, half q
            if l == 0:
                return w1a16[:, 256 * q:256 * (q + 1)]
            return wa_t[:, 512 * kc + 256 * q:512 * kc + 256 * q + 256]

        def wb_slice(fi, fo):
            # [128, 128] slice: wb[:, 512*fi + 128*fo]
            return wb_t[:, 512 * fi + 128 * fo:512 * fi + 128 * fo + 128]

        # z = h @ Wa  (halves in separate psum banks)
        ba_l = ba_sb[0 if l == 0 else 1]
        z_sb = tmp.tile([72, H], f16, tag='z_sb')
        z_h = [pbig.tile([72, H // 2], f32, tag='pbig', name=f'z{q}')
               for q in range(2)]
        for q in range(2):
            for c in range(nk):
                cs = 32 if l == 0 else 128
                nc.tensor.matmul(z_h[q][:, :],
                                 hT_chunks[c][0:cs, 0:72], wa_slice(c, q),
                                 start=(c == 0), stop=(c == nk - 1))
        nc.vector.tensor_copy(z_sb[:, 0:H // 2], z_h[0][:, :])
        nc.scalar.copy(z_sb[:, H // 2:], z_h[1][:, :])

        # u = Mp @ z + ba (rank-1 closer); halves in separate banks
        u_h = [pbig.tile([72, H // 2], f32, tag='pbig', name=f'u{q}')
               for q in range(2)]
        for q in range(2):
            nc.tensor.matmul(u_h[q][:, :], m72x[:, :],
                             z_sb[:, q * (H // 2):(q + 1) * (H // 2)],
                             start=True, stop=False)
        for q in range(2):
            nc.tensor.matmul(u_h[q][:, :], ones72[:, :],
                             ba_l[:, q * (H // 2):(q + 1) * (H // 2)],
                             start=False, stop=True)

        # LN stats: one bn_stats per half, aggregated together
        bst = tmp.tile([72, 12], f32, tag='bst')
        nc.vector.bn_stats(bst[:, 0:6], u_h[0][:, :])
        nc.vector.bn_stats(bst[:, 6:12], u_h[1][:, :])
        mv = tmp.tile([72, 2], f32, tag='mv')
        nc.vector.bn_aggr(mv[:, :], bst[:, :])
        std = tmp.tile([72, 1], f32, tag='std')
        nc.scalar.activation(std[:, :], mv[:, 1:2], AF.Sqrt,
                             bias=eps[:, 0:1])
        rstd = tmp.tile([72, 1], f32, tag='rstd')
        nc.vector.reciprocal(rstd[:, :], std[:, :])
        mb = tmp.tile([72, 1], f32, tag='mb')  # -mean*rstd
        nc.vector.scalar_tensor_tensor(mb[:, :], mv[:, 0:1], -1.0,
                                       rstd[:, 0:1], AL.mult, AL.mult)

        # us = (u - mean) * rstd -> fp16 (DVE half / ACT half), then
        # rT chunk = relu(us.T * g + be): PE transpose + per-chunk scale
        us = tmp.tile([72, H], f16, tag='us')
        nc.vector.tensor_scalar(us[:, 0:H // 2], u_h[0][:, :],
                                mv[:, 0:1], rstd[:, 0:1],
                                AL.subtract, AL.mult)
        nc.scalar.activation(us[:, H // 2:], u_h[1][:, :], AF.Identity,
                             bias=mb[:, 0:1], scale=rstd[:, 0:1])
        rT_chunks = []
        for c in range(4):
            tp = psm.tile([128, 72], f16, tag='psm')
            nc.tensor.transpose(tp[:, :], us[:, 128 * c:128 * (c + 1)],
                                eye72)
            rt = tmp.tile([128, 72], f16, tag=f'rT{c}', name=f'rT{c}')
            g_c = fv[:, gcol + c:gcol + c + 1]
            be_c = fv[:, becol + c:becol + c + 1]
            nc.scalar.activation(rt[:, :], tp[:, :], AF.Relu,
                                 bias=be_c, scale=g_c)
            rT_chunks.append(rt)

        # vT chunks = Wb-chunk.T @ rT-chunk (feature-major; two psum banks)
        vt_ps = [pbig.tile([128, 2 * 72], f32, tag='pvt', name=f'vt{q}')
                 for q in range(2)]
        for fi in range(4):
            for fo in range(4):
                q, o = fo % 2, fo // 2
                nc.tensor.matmul(
                    vt_ps[q][:, 72 * o:72 * (o + 1)],
                    wb_slice(fi, fo), rT_chunks[fi][:, :],
                    start=(fi == 0 and fo < 2), stop=(fi == 3),
                    skip_group_check=True)
        new_chunks = []
        pf = tmp.tile([128, 8], f32, tag='pf')
        for fo in range(4):
            q, o = fo % 2, fo // 2
            hn = tmp.tile([128, 72], f16, tag=f'hnT{fo}', name=f'hnT{fo}')
            srcp = vt_ps[q][:, 72 * o:72 * (o + 1)]
            bb = fv[:, bbtcol + fo:bbtcol + fo + 1]
            if fo < 2:
                nc.vector.tensor_scalar_add(hn[:, :], srcp, bb[:, 0:1])
            else:
                nc.scalar.activation(hn[:, :], srcp, AF.Identity,
                                     bias=bb[:, 0:1])
            new_chunks.append(hn)
            # pooling: free-dim reduces per (chunk, graph)
            for g in range(2):
                nc.vector.tensor_reduce(
                    pf[:, 2 * fo + g:2 * fo + g + 1],
                    hn[:, 36 * g:36 * g + 36],
                    mybir.AxisListType.X, AL.add)
        nc.vector.tensor_copy(featsT_l[l][:, :], pf[:, :])

        # queue this layer's gi matmuls (flushed after layer 3)
        def make_gi(kcv, lv):
            def emit_gi(is_first, is_last):
                for j in range(12):
                    nc.tensor.matmul(
                        gi_ps[:, 2 * j:2 * j + 2],
                        wiht_chunk(kcv, j),
                        featsT_l[lv][:, 2 * (kcv - 4 * lv):
                                     2 * (kcv - 4 * lv) + 2],
                        start=(is_first and j == 0), stop=is_last,
                        skip_group_check=True)
            return emit_gi
        for mc in range(4):
            gi_backlog[4 * l + mc] = make_gi(4 * l + mc, l)
        hT_chunks = new_chunks

    gi_order = list(range(12))
    for i, kc in enumerate(gi_order):
        gi_backlog[kc](i == 0, i == 11)
    gi_backlog = {}
    gin_ctx.close()  # free GIN psum banks for the GRU pools
    pgru = ctx.enter_context(tc.tile_pool(name="pgru", bufs=2, space="PSUM"))

    # ---- GRU setup ----
    # gib2[:, 2j+g]: r,z biases (z negated) combined with gi; n cols are
    # gi+bih only (bhh_n lives in fv[:,24:28])
    gib2 = act.tile([128, 24], f32, tag='gib2')
    nc.vector.tensor_tensor(gib2[:, :], gi_ps[:, :], fv[:, 0:24], AL.add)

    def gib_r(g):
        return gib2[:, g:8:2]

    def gib_zn(g):
        return gib2[:, 8 + g:16:2]

    def gib_n(g):
        return gib2[:, 16 + g:24:2]

    # f16 r/zn gate biases, contiguous per (gate, phase), for the
    # group-opening eye128 matmuls
    gib16 = act.tile([128, 16], f16, tag='gib16')
    for g in range(2):
        nc.vector.tensor_copy(gib16[:, 8 * g:8 * g + 4], gib_r(g))
        nc.vector.tensor_copy(gib16[:, 8 * g + 4:8 * g + 8], gib_zn(g))

    # ---- GRU steps ----
    # step 0: h=0 so gr=0; gates come straight from gib2
    g0 = 0 if L > 1 else 1
    r0 = tmp.tile([128, 4], f32, tag='r')
    nc.scalar.activation(r0[:, :], gib_r(g0), AF.Sigmoid)
    w0 = tmp.tile([128, 4], f32, tag='w')
    nc.scalar.activation(w0[:, :], gib_zn(g0), AF.Sigmoid)
    nt = tmp.tile([128, 4], f32, tag='nt')
    nc.vector.tensor_tensor(nt[:, :], r0[:, :], fv[:, 24:28], AL.mult)
    nc.vector.tensor_tensor(nt[:, :], nt[:, :], gib_n(g0), AL.add)
    n0 = tmp.tile([128, 4], f32, tag='n')
    nc.scalar.activation(n0[:, :], nt[:, :], AF.Tanh)
    h_b = tmp.tile([128, 4], f16, tag='h_b')
    nc.vector.tensor_tensor(h_b[:, :], w0[:, :], n0[:, :], AL.mult)

    for t in range(1, L):
        gs = 0 if t < L - 1 else 1
        # psum tiles pre-initialized with gate biases by gpsimd
        pr = pgru.tile([128, 4], f32, tag='pr', name=f'pr{t}')
        pn = pgru.tile([128, 4], f32, tag='pn', name=f'pn{t}')
        pz = pgru.tile([128, 4], f32, tag='pz', name=f'pz{t}')
        # burst order: n, r, z; each group opened by an eye128 matmul
        # that deposits the gate bias
        for out_ps, js, brhs in (
                (pn, range(8, 12), bhn16[:, :]),
                (pr, range(0, 4), gib16[:, 8 * gs:8 * gs + 4]),
                (pz, range(4, 8), gib16[:, 8 * gs + 4:8 * gs + 8])):
            j0 = js[0]
            nc.tensor.matmul(out_ps[:, :], eye128[:, :], brhs,
                             start=True, stop=False, skip_group_check=True)
            for j in js:
                for c in range(4):
                    nc.tensor.matmul(
                        out_ps[:, j - j0:j - j0 + 1],
                        whht_chunk(c, j), h_b[:, c:c + 1],
                        start=False, stop=(j == js[-1] and c == 3),
                        skip_group_check=True)

        r = tmp.tile([128, 4], f32, tag='r')
        nc.scalar.activation(r[:, :], pr[:, :], AF.Sigmoid)
        m = tmp.tile([128, 4], f32, tag='m')
        nc.vector.tensor_tensor(m[:, :], r[:, :], pn[:, :], AL.mult)
        nt = tmp.tile([128, 4], f32, tag='nt')
        nc.vector.tensor_tensor(nt[:, :], m[:, :], gib_n(gs), AL.add)
        n = tmp.tile([128, 4], f32, tag='n')
        nc.scalar.activation(n[:, :], nt[:, :], AF.Tanh)
        a = tmp.tile([128, 4], f32, tag='a')
        nc.vector.tensor_tensor(a[:, :], n[:, :], h_b[:, :], AL.subtract)
        w = tmp.tile([128, 4], f32, tag='w')
        nc.scalar.activation(w[:, :], pz[:, :], AF.Sigmoid)
        wm = tmp.tile([128, 4], f32, tag='wm')
        nc.vector.tensor_tensor(wm[:, :], w[:, :], a[:, :], AL.mult)
        h_new = tmp.tile([128, 4], f16, tag='h_b')
        nc.vector.tensor_tensor(h_new[:, :], wm[:, :], h_b[:, :], AL.add)
        h_b = h_new

    # ---- classifier ----
    hid_ps = pgi.tile([128, 8], f32, tag='gi', name='hid')
    nc.tensor.matmul(hid_ps[:, :], eye128[:, :], bc116[:, :],
                     start=True, stop=False, skip_group_check=True)
    for mc in range(8):
        for c in range(4):
            t_q = wc1t[c // 2]
            base = 1024 * (c % 2) + 128 * mc
            nc.tensor.matmul(
                hid_ps[:, mc:mc + 1], t_q[:, base:base + 128],
                h_b[:, c:c + 1], start=False,
                stop=(mc == 7 and c == 3), skip_group_check=True)
    hid = tmp.tile([128, 8], f16, tag='hid_sb')
    nc.scalar.activation(hid[:, :], hid_ps[:, :], AF.Relu)
    fin_ps = pgru.tile([1, 1], f32, tag='pr', name='fin')
    for mc in range(8):
        nc.tensor.matmul(fin_ps[:, :], hid[:, mc:mc + 1], wc2[:, mc:mc + 1],
                         start=(mc == 0), stop=(mc == 7))
    out_sb = tmp.tile([1, 1], f32, tag='out_sb')
    nc.scalar.activation(out_sb[:, :], fin_ps[:, :], AF.Identity,
                         bias=fv[0:1, 28:29], scale=1.0)
    nc.sync.dma_start(out_dram, out_sb[:, :])


def _build_program(L, blobs):
    from contextlib import ExitStack
    import concourse.bacc as bacc
    import concourse.tile as tile
    import concourse.mybir as mybir

    nc = bacc.Bacc("TRN2", target_bir_lowering=False, debug=False,
                   num_devices=8)
    d = {}
    for name, arr in blobs.items():
        d[name] = nc.dram_tensor(name, list(arr.shape),
                                 mybir.dt.from_np(arr.dtype),
                                 kind="ExternalInput").ap()
    out_dram = nc.dram_tensor("out", [1], mybir.dt.float32,
                              kind="ExternalOutput").ap()
    with tile.TileContext(nc) as tc:
        with ExitStack() as ctx:
            _emit(ctx, tc, d, out_dram, L)
    nc.compile()
    return nc


def _install_ntff_hook():
    """The agent image's antenv lacks axon_hooks; recreate it so
    run_bass_kernel_spmd(trace=True) can capture NTFF profiles."""
    import sys, types
    try:
        import antenv
        if 'antenv.axon_hooks' in sys.modules:
            return
        mod = types.ModuleType('antenv.axon_hooks')
        mod._hook = None

        def set_axon_ntff_profile_hook(hk):
            mod._hook = hk

        def get_axon_ntff_profile_hook():
            return mod._hook

        mod.set_axon_ntff_profile_hook = set_axon_ntff_profile_hook
        mod.get_axon_ntff_profile_hook = get_axon_ntff_profile_hook
        sys.modules['antenv.axon_hooks'] = mod
        antenv.axon_hooks = mod
        from trn_agent_boot.trn_boot import _ntff_profile_via_ctypes
        so = '/opt/axon/libaxon_pjrt.so'
        if os.path.exists(so):
            mod._hook = _ntff_profile_via_ctypes(so)
    except Exception as e:  # profiling is best-effort
        print(f"ntff hook install failed: {e}")


def kernel(**inputs):
    from concourse.bass_utils import run_bass_kernel_spmd

    blobs, L = _prep_inputs(inputs)
    if L not in _prog_cache:
        _prog_cache[L] = _build_program(L, blobs)
    nc = _prog_cache[L]

    in_maps = [dict(blobs) for _ in range(8)]
    trace = bool(int(os.environ.get('KERNEL_TRACE', '0')))
    if trace:
        _install_ntff_hook()
    res = run_bass_kernel_spmd(nc, in_maps, list(range(8)), trace=trace)
    last_run_info['exec_time_ns'] = res.exec_time_ns
    last_run_info['results'] = res
    return np.asarray(res.results[0]['out'], np.float32).reshape(1)


# revision 72
# speedup vs baseline: 1.0487x; 1.0487x over previous
"""Trainium2 Bass kernel for nn_ExplainerCompatibleGinGru.

Math: the reference pads the batch with 31 zero graphs, splits the node dim
into two 36-node graphs (ad = rows 0:36, dis = rows 36:72), runs 3 GIN layers
with sum-pooling, packs [ad x (L-1), dis] as a GRU sequence per batch
element, and returns out[0] -- which depends ONLY on graph 0 (ad), graph 32
(dis) and L = LOS_batch[0].  So the kernel computes: GIN on the stacked
72-node 2-graph block, an L-step GRU on one sequence, and a tiny classifier.

Runs replicated on all 8 cores.  PE matmuls are fp16 with fp32 PSUM
accumulation; LN/gates run in fp32.

Key structure vs the naive version:
- DMA is issued in priority order across all three DMA-capable queues
  (sync HWDGE, act HWDGE, gpsimd SWDGE) so compute is never weight-starved:
  layer-1 weights first, then layer-2/3, whht (GRU), wiht (gi), wc1 last.
- aggregation is a dense matmul with the (A+I)^T block-diagonal operator;
  the GIN-MLP bias rides along as a 73rd "homogeneous" row of z (ones
  column appended to the aggregation operator) -- no bias matmuls.
- LN apply+transpose fuse: us = (u-mean)*rstd, PE transpose, then
  relu(g*x+be) per chunk split across ACT/DVE/GPSIMD.
- GRU per-step: psum tiles are pre-initialized with the gate biases by
  gpsimd (no closer matmuls); the z-gate weights and biases are negated
  host-side so (1-z) comes straight out of one sigmoid:
    h' = w*(n - h) + h   with w = sigmoid(-(Whh_z h + b_z + gi_z)).
- ACT tables (Rsqrt/Sigmoid/Tanh) are prefetched at t=0.
"""

import os
import numpy as np
import ml_dtypes  # noqa: F401

F16 = np.float16

H = 512
LN_EPS = 1e-5

_prog_cache = {}
last_run_info = {}


def _pack_kchunks(w, ncols):
    """[K, N] weight -> [128, (K//128)*N], chunk kc at cols [N*kc, N*(kc+1))."""
    k, n = w.shape
    assert k % 128 == 0 and n == ncols
    nk = k // 128
    return np.ascontiguousarray(
        w.reshape(nk, 128, n).transpose(1, 0, 2).reshape(128, nk * n))


def _prep_inputs(inputs):
    f32 = np.float32

    def bf(x):
        return np.asarray(x, f32).astype(F16)

    x = np.asarray(inputs['x_embedded'], f32)
    tei = np.asarray(inputs['template_edge_index']).astype(np.int64)
    L = int(np.asarray(inputs['LOS_batch']).reshape(-1)[0])

    A = np.zeros((36, 36), f32)
    np.add.at(A, (tei[1], tei[0]), 1.0)
    Mp = A + np.eye(36, dtype=f32)
    m72x = np.zeros((72, 72), f32)
    m72x[:36, :36] = Mp.T
    m72x[36:, 36:] = Mp.T

    W = {k: np.asarray(v, f32) for k, v in inputs.items()
         if k not in ('x_embedded', 'template_edge_index', 'LOS_batch')}

    # xe0: x0 | eye72 | m72 | w1a (rows 0:32) | ba0 | ba1 (row 0)
    # one [72, 1712] blob = one DMA ring entry on the L1-gating chain
    xe0 = np.zeros((72, 1712), F16)
    xe0[:, 0:32] = bf(x)
    xe0[:, 32:104] = np.eye(72, dtype=F16)
    xe0[:, 104:176] = bf(m72x)
    xe0[0:32, 176:688] = bf(W['W1a'])
    xe0[0, 688:1200] = bf(W['b1a'])
    xe0[0, 1200:1712] = bf(W['bha'])

    # fv layout (f32):
    #  0:24  combo24: [p, 2j+g] = bih[p+128j] (+ bhh[p+128j] for j<8);
    #        z-gate cols (j 4..7) NEGATED
    # 24:28  bhh_n tile; 28 bc2; 29:33 g1T; 33:37 be1T; 37:41 ghT; 41:45 behT
    # 45:49  b1bT; 49:53 bhbT; 53:61 bc1T
    fv = np.zeros((128, 61), f32)
    bih_t = W['bih'].reshape(12, 128).T
    bhh_t = W['bhh'].reshape(12, 128).T
    combo = bih_t.copy()
    combo[:, 0:8] += bhh_t[:, 0:8]
    combo[:, 4:8] *= -1.0  # z gates negated
    fv[:, 0:24:2] = combo
    fv[:, 1:24:2] = combo
    fv[:, 24:28] = bhh_t[:, 8:12]
    fv[:, 28] = W['bc2'][0]
    fv[:, 29:33] = W['g1'].reshape(4, 128).T
    fv[:, 33:37] = W['be1'].reshape(4, 128).T
    fv[:, 37:41] = W['gh'].reshape(4, 128).T
    fv[:, 41:45] = W['beh'].reshape(4, 128).T
    fv[:, 45:49] = W['b1b'].reshape(4, 128).T
    fv[:, 49:53] = W['bhb'].reshape(4, 128).T
    fv[:, 53:61] = W['bc1'].reshape(8, 128).T

    # whht: [128, 4*1536], chunk c at cols [1536c, 1536c+1536), j slice
    # 128j within; z-gate columns (j 4..7) negated for the (1-z) trick
    whht = _pack_kchunks(np.ascontiguousarray(W['Whh'].T), 1536)
    for c in range(4):
        whht[:, 1536 * c + 512:1536 * c + 1024] *= -1.0

    w1b = _pack_kchunks(W['W1b'], H)     # [128, 2048]
    wha = _pack_kchunks(W['Wha'], H)
    whb = _pack_kchunks(W['Whb'], H)
    wiht = _pack_kchunks(np.ascontiguousarray(W['Wih'].T), 1536)  # [128,18432]
    for kc in range(12):  # negate z-gate output cols (j 4..7), as in whht
        wiht[:, 1536 * kc + 512:1536 * kc + 1024] *= -1.0
    wc1 = _pack_kchunks(W['Wc1'], 1024)  # [128, 4096]

    # misc16: wc2 | eye128 | bhn16 | bc116  [128, 148]
    misc16 = np.zeros((128, 148), F16)
    misc16[:, 0:8] = bf(np.ascontiguousarray(W['Wc2'].reshape(8, 128).T))
    misc16[:, 8:136] = np.eye(128, dtype=F16)
    misc16[:, 136:140] = bf(bhh_t[:, 8:12])
    misc16[:, 140:148] = bf(W['bc1'].reshape(8, 128).T)

    blobs = {
        'xe0': xe0,
        'fv': fv,
        'misc16': misc16,
        'w1b16': bf(w1b),
        'wha16': bf(wha),
        'whb16': bf(whb),
        'whht0': bf(whht[:, 0:1536]).copy(),
        'whht1': bf(whht[:, 1536:3072]).copy(),
        'whht2': bf(whht[:, 3072:4608]).copy(),
        'whht3': bf(whht[:, 4608:6144]).copy(),
        'wc1a': bf(wc1[:, 0:2048]).copy(),
        'wc1b': bf(wc1[:, 2048:4096]).copy(),
    }
    for i in range(6):
        blobs[f'wiht{i}'] = bf(wiht[:, 3072 * i:3072 * (i + 1)]).copy()
    return blobs, L


def _emit(ctx, tc, d, out_dram, L):
    import concourse.mybir as mybir
    nc = tc.nc
    f32 = mybir.dt.float32
    f16 = mybir.dt.float16
    AF = mybir.ActivationFunctionType
    AL = mybir.AluOpType

    wts = ctx.enter_context(tc.tile_pool(name="wts", bufs=1))
    act = ctx.enter_context(tc.tile_pool(name="act", bufs=1))
    tmp = ctx.enter_context(tc.tile_pool(name="tmp", bufs=2))
    from contextlib import ExitStack
    pgi = ctx.enter_context(tc.tile_pool(name="pgi", bufs=1, space="PSUM"))
    gin_ctx = ctx.enter_context(ExitStack())
    pbig = gin_ctx.enter_context(tc.tile_pool(name="pbig", bufs=2,
                                              space="PSUM"))
    psm = gin_ctx.enter_context(tc.tile_pool(name="psm", bufs=2,
                                             space="PSUM"))

    # ---- DMA issue, priority order across the 3 queues ----------------
    def T(pool, name, shape, dtype=f16):
        t = pool.tile(list(shape), dtype, tag=name, name=name)
        return t

    # Ring model: ~2us fixed cost per entry, FIFO per ring, 3 rings
    # (sync HWDGE / act HWDGE / gpsimd SWDGE) running concurrently.
    # act ring gets exactly TWO entries pushed up-front (a deeper list
    # could stall the ACT sequencer on ring-full and block compute).
    xe0 = T(wts, 'xe0', (72, 1712))
    nc.sync.dma_start(xe0[:, :], d['xe0'])
    fv = T(wts, 'fv', (128, 61), f32)
    nc.sync.dma_start(fv[:, :], d['fv'])
    misc16 = T(wts, 'misc16', (128, 148))
    nc.sync.dma_start(misc16[:, :], d['misc16'])
    m72x = xe0[:, 104:176]
    w1a16 = xe0[0:32, 176:688]
    ba_sb = [xe0[0:1, 688:1200], xe0[0:1, 1200:1712]]
    wc2 = misc16[:, 0:8]
    eye128 = misc16[:, 8:136]
    bhn16 = misc16[:, 136:140]
    bc116 = misc16[:, 140:148]
    ones72 = act.tile([1, 72], f16, tag='ones72')
    nc.vector.memset(ones72[:, :], 1.0)

    # GIN MLP weights: all on the fast gpsimd ring (3 entries, in the
    # order the layers consume them)
    w1b16 = T(wts, 'w1b16', (128, 2048))
    wha16 = T(wts, 'wha16', (128, 2048))
    whb16 = T(wts, 'whb16', (128, 2048))
    nc.gpsimd.dma_start(w1b16[:, :], d['w1b16'])
    nc.gpsimd.dma_start(wha16[:, :], d['wha16'])
    nc.gpsimd.dma_start(whb16[:, :], d['whb16'])

    # sync ring: two wiht chunks early (gi gates the GRU start), then
    # whht halves; gpsimd carries the rest
    wiht_t = [T(wts, f'wiht{i}', (128, 3072)) for i in range(6)]
    whht_t = [T(wts, f'whht{c}', (128, 1536)) for c in range(4)]
    nc.sync.dma_start(wiht_t[0][:, :], d['wiht0'])
    nc.sync.dma_start(wiht_t[3][:, :], d['wiht3'])
    nc.sync.dma_start(whht_t[0][:, :], d['whht0'])
    nc.sync.dma_start(whht_t[2][:, :], d['whht2'])
    nc.gpsimd.dma_start(wiht_t[1][:, :], d['wiht1'])
    nc.gpsimd.dma_start(wiht_t[4][:, :], d['wiht4'])
    nc.gpsimd.dma_start(whht_t[1][:, :], d['whht1'])
    nc.gpsimd.dma_start(whht_t[3][:, :], d['whht3'])
    nc.scalar.dma_start(wiht_t[2][:, :], d['wiht2'])
    nc.scalar.dma_start(wiht_t[5][:, :], d['wiht5'])

    wc1t = [T(wts, 'wc1a', (128, 2048)), T(wts, 'wc1b', (128, 2048))]
    nc.sync.dma_start(wc1t[0][:, :], d['wc1a'])
    nc.gpsimd.dma_start(wc1t[1][:, :], d['wc1b'])

    def wiht_chunk(kc, j):
        q, r = divmod(kc, 2)
        return wiht_t[q][:, 1536 * r + 128 * j:1536 * r + 128 * j + 128]

    def whht_chunk(c, j):
        return whht_t[c][:, 128 * j:128 * (j + 1)]

    # ---- ACT table prefetches (Rsqrt, Sigmoid, Tanh) ------------------
    sc1 = act.tile([1, 1], f32, tag='sc1')
    nc.vector.memset(sc1[:, :], 1.0)
    eps = act.tile([72, 1], f32, tag='eps')
    nc.vector.memset(eps[:, :], LN_EPS)
    for i, af in enumerate((AF.Identity, AF.Sqrt, AF.Sigmoid, AF.Tanh)):
        scd = act.tile([1, 1], f32, tag=f'scd{i}')
        nc.scalar.activation(scd[:, :], sc1[:, :], af)

    x0s = xe0[:, 0:32]
    eye72 = xe0[:, 32:104]

    # per-layer feats tiles: finer deps let layer-l gi matmuls run as
    # soon as layer l's pooling (and the wiht chunk) lands, instead of
    # waiting for all three layers
    featsT_l = [act.tile([128, 8], f16, tag=f'featsT{l}', name=f'featsT{l}')
                for l in range(3)]
    gi_ps = pgi.tile([128, 24], f32, tag='gi')

    # ---- GIN layers (activations live feature-major between layers) ----
    x0T = tmp.tile([32, 72], f16, tag='x0T')
    tp0 = psm.tile([128, 72], f16, tag='psm')
    nc.tensor.transpose(tp0[0:32, :], x0s, eye72)
    nc.vector.tensor_copy(x0T[:, :], tp0[0:32, :])

    gi_backlog = {}
    hT_chunks = [x0T]   # list of [<=128, 72] feature-major chunks
    for l in range(3):
        wa_t = w1a16 if l == 0 else wha16
        wb_t = w1b16 if l == 0 else whb16
        gcol = 29 if l == 0 else 37
        becol = 33 if l == 0 else 41
        bbtcol = 45 if l == 0 else 49
        nk = len(hT_chunks)

        def wa_slice(kc, q):
            # [128, 256] slice for k-chunk kc, half q
            if l == 0:
                return w1a16[:, 256 * q:256 * (q + 1)]
            return wa_t[:, 512 * kc + 256 * q:512 * kc + 256 * q + 256]

        def wb_slice(fi, fo):
            # [128, 128] slice: wb[:, 512*fi + 128*fo]
            return wb_t[:, 512 * fi + 128 * fo:512 * fi + 128 * fo + 128]

        # z = h @ Wa  (halves in separate psum banks)
        ba_l = ba_sb[0 if l == 0 else 1]
        z_sb = tmp.tile([72, H], f16, tag='z_sb')
        z_h = [pbig.tile([72, H // 2], f32, tag='pbig', name=f'z{q}')
               for q in range(2)]
        for q in range(2):
            for c in range(nk):
                cs = 32 if l == 0 else 128
                nc.tensor.matmul(z_h[q][:, :],
                                 hT_chunks[c][0:cs, 0:72], wa_slice(c, q),
                                 start=(c == 0), stop=(c == nk - 1))
        nc.vector.tensor_copy(z_sb[:, 0:H // 2], z_h[0][:, :])
        nc.scalar.copy(z_sb[:, H // 2:], z_h[1][:, :])

        # u = Mp @ z + ba (rank-1 closer); halves in separate banks
        u_h = [pbig.tile([72, H // 2], f32, tag='pbig', name=f'u{q}')
               for q in range(2)]
        for q in range(2):
            nc.tensor.matmul(u_h[q][:, :], m72x[:, :],
                             z_sb[:, q * (H // 2):(q + 1) * (H // 2)],
                             start=True, stop=False)
        for q in range(2):
            nc.tensor.matmul(u_h[q][:, :], ones72[:, :],
                             ba_l[:, q * (H // 2):(q + 1) * (H // 2)],
                             start=False, stop=True)

        # LN stats: one bn_stats per half, aggregated together
        bst = tmp.tile([72, 12], f32, tag='bst')
        nc.vector.bn_stats(bst[:, 0:6], u_h[0][:, :])
        nc.vector.bn_stats(bst[:, 6:12], u_h[1][:, :])
        mv = tmp.tile([72, 2], f32, tag='mv')
        nc.vector.bn_aggr(mv[:, :], bst[:, :])
        std = tmp.tile([72, 1], f32, tag='std')
        nc.scalar.activation(std[:, :], mv[:, 1:2], AF.Sqrt,
                             bias=eps[:, 0:1])
        rstd = tmp.tile([72, 1], f32, tag='rstd')
        nc.vector.reciprocal(rstd[:, :], std[:, :])
        mb = tmp.tile([72, 1], f32, tag='mb')  # -mean*rstd
        nc.vector.scalar_tensor_tensor(mb[:, :], mv[:, 0:1], -1.0,
                                       rstd[:, 0:1], AL.mult, AL.mult)

        # us = (u - mean) * rstd -> fp16 (DVE half / ACT half), then
        # rT chunk = relu(us.T * g + be): PE transpose + per-chunk scale
        us = tmp.tile([72, H], f16, tag='us')
        nc.vector.tensor_scalar(us[:, 0:H // 2], u_h[0][:, :],
                                mv[:, 0:1], rstd[:, 0:1],
                                AL.subtract, AL.mult)
        nc.scalar.activation(us[:, H // 2:], u_h[1][:, :], AF.Identity,
                             bias=mb[:, 0:1], scale=rstd[:, 0:1])
        rT_chunks = []
        for c in range(4):
            tp = psm.tile([128, 72], f16, tag='psm')
            nc.tensor.transpose(tp[:, :], us[:, 128 * c:128 * (c + 1)],
                                eye72)
            rt = tmp.tile([128, 72], f16, tag=f'rT{c}', name=f'rT{c}')
            g_c = fv[:, gcol + c:gcol + c + 1]
            be_c = fv[:, becol + c:becol + c + 1]
            if c != 2:
                nc.scalar.activation(rt[:, :], tp[:, :], AF.Relu,
                                     bias=be_c, scale=g_c)
            else:
                nc.vector.tensor_scalar(rt[:, :], tp[:, :], g_c, be_c,
                                        AL.mult, AL.add)
                nc.vector.tensor_scalar_max(rt[:, :], rt[:, :], 0.0)
            rT_chunks.append(rt)

        # vT chunks = Wb-chunk.T @ rT-chunk (feature-major; two psum banks)
        vt_ps = [pbig.tile([128, 2 * 72], f32, tag='pvt', name=f'vt{q}')
                 for q in range(2)]
        for fi in range(4):
            for fo in range(4):
                q, o = fo % 2, fo // 2
                nc.tensor.matmul(
                    vt_ps[q][:, 72 * o:72 * (o + 1)],
                    wb_slice(fi, fo), rT_chunks[fi][:, :],
                    start=(fi == 0 and fo < 2), stop=(fi == 3),
                    skip_group_check=True)
        new_chunks = []
        pf = tmp.tile([128, 8], f32, tag='pf')
        for fo in range(4):
            q, o = fo % 2, fo // 2
            hn = tmp.tile([128, 72], f16, tag=f'hnT{fo}', name=f'hnT{fo}')
            srcp = vt_ps[q][:, 72 * o:72 * (o + 1)]
            bb = fv[:, bbtcol + fo:bbtcol + fo + 1]
            if fo < 2:
                nc.vector.tensor_scalar_add(hn[:, :], srcp, bb[:, 0:1])
            else:
                nc.scalar.activation(hn[:, :], srcp, AF.Identity,
                                     bias=bb[:, 0:1])
            new_chunks.append(hn)
            # pooling: free-dim reduces per (chunk, graph)
            for g in range(2):
                nc.vector.tensor_reduce(
                    pf[:, 2 * fo + g:2 * fo + g + 1],
                    hn[:, 36 * g:36 * g + 36],
                    mybir.AxisListType.X, AL.add)
        nc.vector.tensor_copy(featsT_l[l][:, :], pf[:, :])

        # queue this layer's gi matmuls (flushed after layer 3)
        def make_gi(kcv, lv):
            def emit_gi(is_first, is_last):
                for j in range(12):
                    nc.tensor.matmul(
                        gi_ps[:, 2 * j:2 * j + 2],
                        wiht_chunk(kcv, j),
                        featsT_l[lv][:, 2 * (kcv - 4 * lv):
                                     2 * (kcv - 4 * lv) + 2],
                        start=(is_first and j == 0), stop=is_last,
                        skip_group_check=True)
            return emit_gi
        for mc in range(4):
            gi_backlog[4 * l + mc] = make_gi(4 * l + mc, l)
        hT_chunks = new_chunks

    gi_order = list(range(12))
    for i, kc in enumerate(gi_order):
        gi_backlog[kc](i == 0, i == 11)
    gi_backlog = {}
    gin_ctx.close()  # free GIN psum banks for the GRU pools
    pgru = ctx.enter_context(tc.tile_pool(name="pgru", bufs=2, space="PSUM"))

    # ---- GRU setup ----
    # gib2[:, 2j+g]: r,z biases (z negated) combined with gi; n cols are
    # gi+bih only (bhh_n lives in fv[:,24:28])
    gib2 = act.tile([128, 24], f32, tag='gib2')
    nc.vector.tensor_tensor(gib2[:, :], gi_ps[:, :], fv[:, 0:24], AL.add)

    def gib_r(g):
        return gib2[:, g:8:2]

    def gib_zn(g):
        return gib2[:, 8 + g:16:2]

    def gib_n(g):
        return gib2[:, 16 + g:24:2]

    # f16 r/zn gate biases, contiguous per (gate, phase), for the
    # group-opening eye128 matmuls
    gib16 = act.tile([128, 16], f16, tag='gib16')
    for g in range(2):
        nc.vector.tensor_copy(gib16[:, 8 * g:8 * g + 4], gib_r(g))
        nc.vector.tensor_copy(gib16[:, 8 * g + 4:8 * g + 8], gib_zn(g))

    # ---- GRU steps ----
    # step 0: h=0 so gr=0; gates come straight from gib2
    g0 = 0 if L > 1 else 1
    r0 = tmp.tile([128, 4], f32, tag='r')
    nc.scalar.activation(r0[:, :], gib_r(g0), AF.Sigmoid)
    w0 = tmp.tile([128, 4], f32, tag='w')
    nc.scalar.activation(w0[:, :], gib_zn(g0), AF.Sigmoid)
    nt = tmp.tile([128, 4], f32, tag='nt')
    nc.vector.tensor_tensor(nt[:, :], r0[:, :], fv[:, 24:28], AL.mult)
    nc.vector.tensor_tensor(nt[:, :], nt[:, :], gib_n(g0), AL.add)
    n0 = tmp.tile([128, 4], f32, tag='n')
    nc.scalar.activation(n0[:, :], nt[:, :], AF.Tanh)
    h_b = tmp.tile([128, 4], f16, tag='h_b')
    nc.vector.tensor_tensor(h_b[:, :], w0[:, :], n0[:, :], AL.mult)

    for t in range(1, L):
        gs = 0 if t < L - 1 else 1
        # psum tiles pre-initialized with gate biases by gpsimd
        pr = pgru.tile([128, 4], f32, tag='pr', name=f'pr{t}')
        pn = pgru.tile([128, 4], f32, tag='pn', name=f'pn{t}')
        pz = pgru.tile([128, 4], f32, tag='pz', name=f'pz{t}')
        # burst order: n, r, z; each group opened by an eye128 matmul
        # that deposits the gate bias
        for out_ps, js, brhs in (
                (pn, range(8, 12), bhn16[:, :]),
                (pr, range(0, 4), gib16[:, 8 * gs:8 * gs + 4]),
                (pz, range(4, 8), gib16[:, 8 * gs + 4:8 * gs + 8])):
            j0 = js[0]
            nc.tensor.matmul(out_ps[:, :], eye128[:, :], brhs,
                             start=True, stop=False, skip_group_check=True)
            for j in js:
                for c in range(4):
                    nc.tensor.matmul(
                        out_ps[:, j - j0:j - j0 + 1],
                        whht_chunk(c, j), h_b[:, c:c + 1],
                        start=False, stop=(j == js[-1] and c == 3),
                        skip_group_check=True)

        r = tmp.tile([128, 4], f32, tag='r')
        nc.scalar.activation(r[:, :], pr[:, :], AF.Sigmoid)
        m = tmp.tile([128, 4], f32, tag='m')
        nc.vector.tensor_tensor(m[:, :], r[:, :], pn[:, :], AL.mult)
        nt = tmp.tile([128, 4], f32, tag='nt')
        nc.vector.tensor_tensor(nt[:, :], m[:, :], gib_n(gs), AL.add)
        n = tmp.tile([128, 4], f32, tag='n')
        nc.scalar.activation(n[:, :], nt[:, :], AF.Tanh)
        a = tmp.tile([128, 4], f32, tag='a')
        nc.vector.tensor_tensor(a[:, :], n[:, :], h_b[:, :], AL.subtract)
        w = tmp.tile([128, 4], f32, tag='w')
        nc.scalar.activation(w[:, :], pz[:, :], AF.Sigmoid)
        wm = tmp.tile([128, 4], f32, tag='wm')
        nc.vector.tensor_tensor(wm[:, :], w[:, :], a[:, :], AL.mult)
        h_new = tmp.tile([128, 4], f16, tag='h_b')
        nc.vector.tensor_tensor(h_new[:, :], wm[:, :], h_b[:, :], AL.add)
        h_b = h_new

    # ---- classifier ----
    hid_ps = pgi.tile([128, 8], f32, tag='gi', name='hid')
    nc.tensor.matmul(hid_ps[:, :], eye128[:, :], bc116[:, :],
                     start=True, stop=False, skip_group_check=True)
    for mc in range(8):
        for c in range(4):
            t_q = wc1t[c // 2]
            base = 1024 * (c % 2) + 128 * mc
            nc.tensor.matmul(
                hid_ps[:, mc:mc + 1], t_q[:, base:base + 128],
                h_b[:, c:c + 1], start=False,
                stop=(mc == 7 and c == 3), skip_group_check=True)
    hid = tmp.tile([128, 8], f16, tag='hid_sb')
    nc.scalar.activation(hid[:, :], hid_ps[:, :], AF.Relu)
    fin_ps = pgru.tile([1, 1], f32, tag='pr', name='fin')
    for mc in range(8):
        nc.tensor.matmul(fin_ps[:, :], hid[:, mc:mc + 1], wc2[:, mc:mc + 1],
                         start=(mc == 0), stop=(mc == 7))
    out_sb = tmp.tile([1, 1], f32, tag='out_sb')
    nc.scalar.activation(out_sb[:, :], fin_ps[:, :], AF.Identity,
                         bias=fv[0:1, 28:29], scale=1.0)
    nc.sync.dma_start(out_dram, out_sb[:, :])


def _build_program(L, blobs):
    from contextlib import ExitStack
    import concourse.bacc as bacc
    import concourse.tile as tile
    import concourse.mybir as mybir

    nc = bacc.Bacc("TRN2", target_bir_lowering=False, debug=False,
                   num_devices=8)
    d = {}
    for name, arr in blobs.items():
        d[name] = nc.dram_tensor(name, list(arr.shape),
                                 mybir.dt.from_np(arr.dtype),
                                 kind="ExternalInput").ap()
    out_dram = nc.dram_tensor("out", [1], mybir.dt.float32,
                              kind="ExternalOutput").ap()
    with tile.TileContext(nc) as tc:
        with ExitStack() as ctx:
            _emit(ctx, tc, d, out_dram, L)
    nc.compile()
    return nc


def _install_ntff_hook():
    """The agent image's antenv lacks axon_hooks; recreate it so
    run_bass_kernel_spmd(trace=True) can capture NTFF profiles."""
    import sys, types
    try:
        import antenv
        if 'antenv.axon_hooks' in sys.modules:
            return
        mod = types.ModuleType('antenv.axon_hooks')
        mod._hook = None

        def set_axon_ntff_profile_hook(hk):
            mod._hook = hk

        def get_axon_ntff_profile_hook():
            return mod._hook

        mod.set_axon_ntff_profile_hook = set_axon_ntff_profile_hook
        mod.get_axon_ntff_profile_hook = get_axon_ntff_profile_hook
        sys.modules['antenv.axon_hooks'] = mod
        antenv.axon_hooks = mod
        from trn_agent_boot.trn_boot import _ntff_profile_via_ctypes
        so = '/opt/axon/libaxon_pjrt.so'
        if os.path.exists(so):
            mod._hook = _ntff_profile_via_ctypes(so)
    except Exception as e:  # profiling is best-effort
        print(f"ntff hook install failed: {e}")


def kernel(**inputs):
    from concourse.bass_utils import run_bass_kernel_spmd

    blobs, L = _prep_inputs(inputs)
    if L not in _prog_cache:
        _prog_cache[L] = _build_program(L, blobs)
    nc = _prog_cache[L]

    in_maps = [dict(blobs) for _ in range(8)]
    trace = bool(int(os.environ.get('KERNEL_TRACE', '0')))
    if trace:
        _install_ntff_hook()
    res = run_bass_kernel_spmd(nc, in_maps, list(range(8)), trace=trace)
    last_run_info['exec_time_ns'] = res.exec_time_ns
    last_run_info['results'] = res
    return np.asarray(res.results[0]['out'], np.float32).reshape(1)
